# revision 9
# baseline (speedup 1.0000x reference)
"""L2 self-attention (q==k) Bass/Tile kernel for 8 TRN2 NeuronCores.

Sharding: core c = 2*b + g handles batch b and head-group g (8 of 16 heads).
Each core computes the partial output  attn_out_bg @ Wo[g*512:(g+1)*512, :].
Host sums the two partials per batch and adds bo (+ the folded v-bias).

Math per head (s = DIM_HEAD**-0.5):
  sim_ij = -s*||q_i - q_j||^2 = 2s*AB_ij - s*AA_i - s*AA_j
  softmax rows are invariant to per-row constants, so
  P_ij = exp(2s*AB_ij - s*AA_j) / sum_j exp(2s*AB_ij - s*AA_j)
  Exponent is bounded (2ab <= a^2+b^2), no max-subtraction needed.
  out = P @ v ; den folded in as a 65th (ones) column of v.
  bq drops out entirely (L2 distances are shift-invariant); bv commutes
  through the softmax average (rows sum to 1) and is added on the host
  as bv_g @ Wo_g.

v4: all matmul operands 16-bit (fp32 PSUM accumulation).  fp32r moving
operands stream at half rate and keep the PE HAM clock gate at K=4/8
(1.2 GHz); 16-bit streams 1 row/cycle and holds 2.4 GHz.  The -s*AA_j
term is folded into the scores matmul as a 65th contraction row:
stationary qS row 64 = AA_j (ones-vector PE matmul over squared q),
moving qM row 64 = -0.5, exp applies scale 2s.  The scalar engine runs
nothing but the 256 exps (its ~285us is the co-roofline with the PE);
all copies/squares live on vector/gpsimd.  i-chunks are the outer loop
so the first half of the output projection overlaps the second
attention chunk.
"""

import numpy as np

B, N, D = 4, 2048, 1024
HEADS, DIM_HEAD = 16, 64
INNER = HEADS * DIM_HEAD
SCALE = DIM_HEAD ** -0.5

NCORES = 8
NH = 8            # heads per core
DL = NH * DIM_HEAD  # 512 local inner dims
KT = D // 128     # 8 contraction tiles for projections
JT = N // 128     # 16 key tiles
IC = 2            # i-chunks of 1024 columns
ICW = N // IC     # 1024

_CACHE = {}


def _build_nc():
    import concourse.bacc as bacc
    import concourse.bass as bass
    import concourse.mybir as mybir
    import concourse.tile as tile

    f32 = mybir.dt.float32
    bf16 = mybir.dt.bfloat16
    f16 = mybir.dt.float16
    EXP = mybir.ActivationFunctionType.Exp

    nc = bacc.Bacc("TRN2", target_bir_lowering=False, debug=False,
                   num_devices=NCORES)

    xTa_d = nc.dram_tensor("xTa", [D, N], bf16, kind="ExternalInput")
    wqa_d = nc.dram_tensor("wqa", [D, DL], bf16, kind="ExternalInput")
    wva_d = nc.dram_tensor("wva", [D, DL], bf16, kind="ExternalInput")
    wo_d = nc.dram_tensor("wo", [DL, D], bf16, kind="ExternalInput")
    part_d = nc.dram_tensor("part", [N, D], f32, kind="ExternalOutput")
    xTa = xTa_d.ap()
    wqa = wqa_d.ap()
    wva = wva_d.ap()
    wo_ap = wo_d.ap()
    part = part_d.ap()

    with tile.TileContext(nc) as tc, \
         tc.tile_pool(name="persist", bufs=1) as persist:
        # ---- persistent tensors (whole-kernel lifetime) ----
        # per-head stationary q: rows 0..63 = q_h, row 64 = AA_h
        qS = [persist.tile([65, N], bf16, tag=f"qS{h}", name=f"qS{h}")
              for h in range(NH)]
        # per-head moving q: rows 0..63 = q_h, row 64 = -0.5
        qM = [persist.tile([65, N], bf16, tag=f"qM{h}", name=f"qM{h}")
              for h in range(NH)]
        v_aug = persist.tile([128, JT, NH * 65], f16, tag="v_aug", name="v_aug")
        ones64 = persist.tile([64, 1], bf16, tag="ones64", name="ones64")
        nc.vector.memset(ones64, 1.0)

        ones1 = persist.tile([128, JT, 1], f16, tag="ones1", name="ones1")
        nc.vector.memset(ones1, 1.0)
        for h in range(NH):
            nc.vector.tensor_copy(v_aug[:, :, h * 65 + 64 : h * 65 + 65], ones1)
            nc.vector.memset(qM[h][64:65, :], -0.5)

        # ---- phase 1: projections ----
        with tc.tile_pool(name="pin", bufs=1) as pin, \
             tc.tile_pool(name="sqp", bufs=2) as sqp:
            xt = [pin.tile([128, N], bf16, tag=f"xt{k}", name=f"xt{k}") for k in range(KT)]
            wq = [pin.tile([128, DL], bf16, tag=f"wq{k}", name=f"wq{k}") for k in range(KT)]
            wv = [pin.tile([128, DL], bf16, tag=f"wv{k}", name=f"wv{k}") for k in range(KT)]
            for k in range(KT):
                sl = slice(k * 128, (k + 1) * 128)
                nc.sync.dma_start(out=xt[k], in_=xTa[sl, :])
                nc.sync.dma_start(out=wq[k], in_=wqa[sl, :])
                nc.sync.dma_start(out=wv[k], in_=wva[sl, :])

            # q[d, i] per dt tile: lhsT = wqa[:, d-tile], rhs = xTa[:, i-chunk]
            with tc.tile_pool(name="qps", bufs=2, space="PSUM") as qps:
                for dt in range(4):
                    ps = qps.tile([128, N], f32, tag="qproj")
                    dsl = slice(dt * 128, (dt + 1) * 128)
                    for k in range(KT):
                        for nck in range(4):
                            nsl = slice(nck * 512, (nck + 1) * 512)
                            nc.tensor.matmul(ps[:, nsl], lhsT=wq[k][:, dsl],
                                             rhs=xt[k][:, nsl],
                                             start=(k == 0), stop=(k == KT - 1))
                    h0, h1 = 2 * dt, 2 * dt + 1
                    # split heads into per-head stationary/moving tiles (bf16)
                    nc.vector.tensor_copy(qS[h0][0:64, :], ps[0:64, :])
                    nc.vector.tensor_copy(qS[h1][0:64, :], ps[64:128, :])
                    nc.vector.tensor_copy(qM[h0][0:64, :], ps[0:64, :])
                    nc.vector.tensor_copy(qM[h1][0:64, :], ps[64:128, :])

            # v[i, d] : lhsT = xTa[:, i-tile], rhs = wva ; scatter into v_aug
            # AA rows: sq = q_h^2 (from the bf16 qS rows so rounding matches
            # the scores matmul operands), then ones^T @ sq via the PE.
            with tc.tile_pool(name="vps", bufs=4, space="PSUM") as vps, \
                 tc.tile_pool(name="aaps", bufs=4, space="PSUM") as aaps:
                for h in range(NH):
                    sq = sqp.tile([64, N], bf16, tag="sq")
                    nc.vector.tensor_mul(sq, qS[h][0:64, :], qS[h][0:64, :])
                    for c4 in range(4):
                        csl = slice(c4 * 512, (c4 + 1) * 512)
                        aps = aaps.tile([1, 512], f32, tag="aa")
                        nc.tensor.matmul(aps, lhsT=ones64, rhs=sq[:, csl],
                                         start=True, stop=True)
                        nc.vector.tensor_copy(qS[h][64:65, csl], aps)
                for it in range(JT):
                    ps = vps.tile([128, DL], f32, tag="vproj")
                    isl = slice(it * 128, (it + 1) * 128)
                    for k in range(KT):
                        nc.tensor.matmul(ps, lhsT=xt[k][:, isl], rhs=wv[k],
                                         start=(k == 0), stop=(k == KT - 1))
                    src = ps.rearrange("p (h w) -> p h w", w=64)
                    dst = v_aug[:, it, :].rearrange("p (h w) -> p h w", w=65)
                    nc.vector.tensor_copy(dst[:, :, 0:64], src)

        # allocated after the projection pool closes so phase-1 SBUF peak
        # (xt/wq/wv tiles) and these never coexist in the address map
        p2 = tc.alloc_tile_pool(name="persist2", bufs=1)
        ot = [p2.tile([128, N], bf16, tag=f"ot{t}", name=f"ot{t}")
              for t in range(4)]
        wo_sb = [p2.tile([128, D], bf16, tag=f"wo{t}", name=f"wo{t}")
                 for t in range(4)]
        for t in range(4):
            nc.sync.dma_start(out=wo_sb[t], in_=wo_ap[t * 128 : (t + 1) * 128, :])

        # ---- phase 2+3: attention (ic outer) with overlapped output proj ----
        # scores K=65: sp = q_h^T q_h - AA_j/2 ; exp(2s*sp) is the softmax
        # numerator with the j-bias folded in.
        # Output projection for an i-range runs as soon as all heads finish
        # that ic chunk, reusing the nps PSUM ring (same 4KB/partition tiles).
        with tc.tile_pool(name="sps", bufs=2, space="PSUM") as sps, \
             tc.tile_pool(name="nps", bufs=2, space="PSUM") as nps, \
             tc.tile_pool(name="gp", bufs=3) as gp, \
             tc.tile_pool(name="nrm", bufs=2) as nrm, \
             tc.tile_pool(name="osb", bufs=2) as osb:
            for ic in range(IC):
                i0 = ic * ICW
                for h in range(NH):
                    dt, half = divmod(h, 2)
                    rows = slice(half * 64, half * 64 + 64)
                    vsl = slice(h * 65, (h + 1) * 65)
                    nm = nps.tile([65, ICW], f32, tag="num")
                    gs = [None] * JT
                    for jt in range(JT):
                        jsl = slice(jt * 128, (jt + 1) * 128)
                        sp = sps.tile([128, ICW], f32, tag="scores")
                        for q in range(2):
                            qsl = slice(q * 512, (q + 1) * 512)
                            nc.tensor.matmul(
                                sp[:, qsl], lhsT=qS[h][:, jsl],
                                rhs=qM[h][:, i0 + q * 512 : i0 + (q + 1) * 512],
                                start=True, stop=True)
                        g = gp.tile([128, ICW], f16, tag="gtile")
                        nc.scalar.activation(out=g, in_=sp, func=EXP,
                                             scale=2.0 * SCALE)
                        gs[jt] = g
                        # one-step software skew: num(jt-1) after S(jt)/exp(jt)
                        if jt > 0:
                            for q in range(2):
                                qsl = slice(q * 512, (q + 1) * 512)
                                nc.tensor.matmul(nm[:, qsl], lhsT=v_aug[:, jt - 1, vsl],
                                                 rhs=gs[jt - 1][:, qsl],
                                                 start=(jt == 1), stop=False)
                    for q in range(2):
                        qsl = slice(q * 512, (q + 1) * 512)
                        nc.tensor.matmul(nm[:, qsl], lhsT=v_aug[:, JT - 1, vsl],
                                         rhs=gs[JT - 1][:, qsl],
                                         start=False, stop=True)
                    # normalize: ot[rows, i0:i0+ICW] = nm[0:64] / nm[64]
                    # (reciprocal_approx_fast misreads PSUM - stage via SBUF)
                    dsb = nrm.tile([1, ICW], f32, tag="dsb", name="dsb")
                    nc.vector.tensor_copy(dsb, nm[64:65, :])
                    rd = nrm.tile([1, ICW], f32, tag="rden", name="rden")
                    nc.vector.reciprocal_approx_fast(out=rd, in_=dsb)
                    rdb = nrm.tile([64, ICW], f32, tag="rdenb", name="rdenb")
                    nc.gpsimd.partition_broadcast(rdb, rd)
                    nc.vector.tensor_mul(ot[dt][rows, i0 : i0 + ICW],
                                         nm[0:64, :], rdb)

                # output projection for the completed i-range
                for it in range(ic * JT // IC, (ic + 1) * JT // IC):
                    isl = slice(it * 128, (it + 1) * 128)
                    # same tag+shape as the scores tiles: shares the sps PSUM
                    # ring (adding a tag would double the pool footprint)
                    ps = sps.tile([128, 1024], f32, tag="scores")
                    for ock in range(2):
                        osl = slice(ock * 512, (ock + 1) * 512)
                        for dlt in range(4):
                            nc.tensor.matmul(ps[:, osl], lhsT=ot[dlt][:, isl],
                                             rhs=wo_sb[dlt][:, osl],
                                             start=(dlt == 0), stop=(dlt == 3))
                    ob = osb.tile([128, 1024], f32, tag="obuf", name="obuf")
                    nc.vector.tensor_copy(ob, ps)
                    nc.sync.dma_start(out=part[isl, :], in_=ob)

        p2.release()

    nc.compile()
    return nc


def _get_nc():
    if "nc" not in _CACHE:
        _CACHE["nc"] = _build_nc()
    return _CACHE["nc"]


def make_in_maps(x, Wq, bq, Wv, bv, Wo, bo):
    from ml_dtypes import bfloat16

    x = np.asarray(x, dtype=np.float32)
    Wq = np.asarray(Wq, dtype=np.float32)
    Wv = np.asarray(Wv, dtype=np.float32)
    Wo = np.asarray(Wo, dtype=np.float32)
    in_maps = []
    for c in range(NCORES):
        b, g = divmod(c, 2)
        gsl = slice(g * DL, (g + 1) * DL)
        in_maps.append({
            "xTa": np.ascontiguousarray(x[b].T).astype(bfloat16),
            "wqa": np.ascontiguousarray(Wq[:, gsl]).astype(bfloat16),
            "wva": np.ascontiguousarray(Wv[:, gsl]).astype(bfloat16),
            "wo": np.ascontiguousarray(Wo[gsl, :]).astype(bfloat16),
        })
    return in_maps


def combine_parts(parts, bv, Wo, bo):
    bo = np.asarray(bo, dtype=np.float32)
    bv = np.asarray(bv, dtype=np.float32)
    Wo = np.asarray(Wo, dtype=np.float32)
    bias = bo + bv @ Wo  # v-bias commutes through the softmax average
    out = np.empty((B, N, D), np.float32)
    for b in range(B):
        out[b] = parts[2 * b] + parts[2 * b + 1] + bias
    return out


def kernel(x, Wq, bq, Wv, bv, Wo, bo):
    from concourse.bass_utils import run_bass_kernel_spmd

    nc = _get_nc()
    in_maps = make_in_maps(x, Wq, bq, Wv, bv, Wo, bo)
    res = run_bass_kernel_spmd(nc, in_maps, core_ids=list(range(NCORES)))
    parts = [r["part"] for r in res.results]
    return combine_parts(parts, bv, Wo, bo)


# revision 10
# speedup vs baseline: 1.0587x; 1.0587x over previous
"""L2 self-attention (q==k) Bass/Tile kernel for 8 TRN2 NeuronCores.

Sharding: core c = 2*b + g handles batch b and head-group g (8 of 16 heads).
Each core computes the partial output  attn_out_bg @ Wo[g*512:(g+1)*512, :].
Host sums the two partials per batch and adds bo (+ the folded v-bias).

Math per head (s = DIM_HEAD**-0.5):
  sim_ij = -s*||q_i - q_j||^2 = 2s*AB_ij - s*AA_i - s*AA_j
  softmax rows are invariant to per-row constants, so
  P_ij = exp(2s*AB_ij - s*AA_j) / sum_j exp(2s*AB_ij - s*AA_j)
  Exponent is bounded (2ab <= a^2+b^2), no max-subtraction needed.
  out = P @ v ; den folded in as a 65th (ones) column of v.
  bq drops out entirely (L2 distances are shift-invariant); bv commutes
  through the softmax average (rows sum to 1) and is added on the host
  as bv @ Wo.

v5: all matmul operands 16-bit (fp32 PSUM accumulation).  fp32r moving
operands stream at half rate and keep the PE HAM clock gate at K=4/8
(1.2 GHz); 16-bit streams 1 row/cycle and holds 2.4 GHz.  The -s*AA_j
term is folded into the scores matmul as a 65th contraction row:
stationary qS row 64 = AA_j (ones-vector PE matmul over squared q),
moving qM row 64 = -0.5, exp applies scale 2s.  AA work is interleaved
per projection d-tile (on the otherwise-idle scalar engine) so head 0's
attention starts ~25us in.  i-chunks are the outer attention loop and
each chunk's output projection is spread through the next chunk's first
head (sharing the scores PSUM ring slot-for-slot), leaving only the
final 8 i-tiles as an un-overlapped tail.
"""

import numpy as np

B, N, D = 4, 2048, 1024
HEADS, DIM_HEAD = 16, 64
INNER = HEADS * DIM_HEAD
SCALE = DIM_HEAD ** -0.5

NCORES = 8
NH = 8            # heads per core
DL = NH * DIM_HEAD  # 512 local inner dims
KT = D // 128     # 8 contraction tiles for projections
JT = N // 128     # 16 key tiles
IC = 2            # i-chunks of 1024 columns
ICW = N // IC     # 1024

_CACHE = {}


def _build_nc():
    import concourse.bacc as bacc
    import concourse.bass as bass
    import concourse.mybir as mybir
    import concourse.tile as tile

    f32 = mybir.dt.float32
    bf16 = mybir.dt.bfloat16
    f16 = mybir.dt.float16
    EXP = mybir.ActivationFunctionType.Exp
    SQUARE = mybir.ActivationFunctionType.Square
    COPY = mybir.ActivationFunctionType.Copy

    nc = bacc.Bacc("TRN2", target_bir_lowering=False, debug=False,
                   num_devices=NCORES)

    xTa_d = nc.dram_tensor("xTa", [D, N], bf16, kind="ExternalInput")
    wqa_d = nc.dram_tensor("wqa", [D, DL], bf16, kind="ExternalInput")
    wva_d = nc.dram_tensor("wva", [D, DL], bf16, kind="ExternalInput")
    wo_d = nc.dram_tensor("wo", [DL, D], bf16, kind="ExternalInput")
    part_d = nc.dram_tensor("part", [N, D], f32, kind="ExternalOutput")
    xTa = xTa_d.ap()
    wqa = wqa_d.ap()
    wva = wva_d.ap()
    wo_ap = wo_d.ap()
    part = part_d.ap()

    with tile.TileContext(nc) as tc, \
         tc.tile_pool(name="persist", bufs=1) as persist:
        # ---- persistent tensors (whole-kernel lifetime) ----
        # per-head stationary q: rows 0..63 = q_h, row 64 = AA_h
        qS = [persist.tile([65, N], bf16, tag=f"qS{h}", name=f"qS{h}")
              for h in range(NH)]
        # per-head moving q: rows 0..63 = q_h, row 64 = -0.5
        qM = [persist.tile([65, N], bf16, tag=f"qM{h}", name=f"qM{h}")
              for h in range(NH)]
        v_aug = persist.tile([128, JT, NH * 65], f16, tag="v_aug", name="v_aug")
        ones64 = persist.tile([64, 1], bf16, tag="ones64", name="ones64")
        nc.vector.memset(ones64, 1.0)

        ones1 = persist.tile([128, JT, 1], f16, tag="ones1", name="ones1")
        nc.vector.memset(ones1, 1.0)
        for h in range(NH):
            nc.vector.tensor_copy(v_aug[:, :, h * 65 + 64 : h * 65 + 65], ones1)
            nc.vector.memset(qM[h][64:65, :], -0.5)

        # ---- phase 1: projections ----
        with tc.tile_pool(name="pin", bufs=1) as pin, \
             tc.tile_pool(name="sqp", bufs=2) as sqp:
            xt = [pin.tile([128, N], bf16, tag=f"xt{k}", name=f"xt{k}") for k in range(KT)]
            wq = [pin.tile([128, DL], bf16, tag=f"wq{k}", name=f"wq{k}") for k in range(KT)]
            wv = [pin.tile([128, DL], bf16, tag=f"wv{k}", name=f"wv{k}") for k in range(KT)]
            for k in range(KT):
                sl = slice(k * 128, (k + 1) * 128)
                nc.sync.dma_start(out=xt[k], in_=xTa[sl, :])
                nc.sync.dma_start(out=wq[k], in_=wqa[sl, :])
                nc.sync.dma_start(out=wv[k], in_=wva[sl, :])

            # q[d, i]: lhsT = wqa[:, d-tile], rhs = xTa[:, i-chunk].
            # AA rows interleaved per dt: sq = q_h^2 (scalar, from the bf16
            # qS rows so rounding matches the scores matmul operands), then
            # ones^T @ sq via the PE, so head0 attention can start early.
            with tc.tile_pool(name="qps", bufs=2, space="PSUM") as qps, \
                 tc.tile_pool(name="aaps", bufs=4, space="PSUM") as aaps:
                for dt in range(4):
                    h0, h1 = 2 * dt, 2 * dt + 1
                    dsl = slice(dt * 128, (dt + 1) * 128)
                    for ich in range(2):
                        ps = qps.tile([128, ICW], f32, tag="qproj")
                        for k in range(KT):
                            for nck in range(2):
                                gco = ich * ICW + nck * 512
                                nc.tensor.matmul(
                                    ps[:, nck * 512 : (nck + 1) * 512],
                                    lhsT=wq[k][:, dsl],
                                    rhs=xt[k][:, gco : gco + 512],
                                    start=(k == 0), stop=(k == KT - 1))
                        csl = slice(ich * ICW, (ich + 1) * ICW)
                        nc.vector.tensor_copy(qS[h0][0:64, csl], ps[0:64, :])
                        nc.vector.tensor_copy(qS[h1][0:64, csl], ps[64:128, :])
                        nc.scalar.activation(qM[h0][0:64, csl], ps[0:64, :], COPY)
                        nc.scalar.activation(qM[h1][0:64, csl], ps[64:128, :], COPY)
                    for hh in (h0, h1):
                        sq = sqp.tile([64, N], bf16, tag="sq")
                        nc.scalar.activation(sq, qS[hh][0:64, :], SQUARE)
                        for c4 in range(4):
                            csl4 = slice(c4 * 512, (c4 + 1) * 512)
                            aps = aaps.tile([1, 512], f32, tag="aa")
                            nc.tensor.matmul(aps, lhsT=ones64, rhs=sq[:, csl4],
                                             start=True, stop=True)
                            nc.vector.tensor_copy(qS[hh][64:65, csl4], aps)

            # v[i, d] : lhsT = xTa[:, i-tile], rhs = wva ; scatter into v_aug
            with tc.tile_pool(name="vps", bufs=4, space="PSUM") as vps:
                for it in range(JT):
                    ps = vps.tile([128, DL], f32, tag="vproj")
                    isl = slice(it * 128, (it + 1) * 128)
                    for k in range(KT):
                        nc.tensor.matmul(ps, lhsT=xt[k][:, isl], rhs=wv[k],
                                         start=(k == 0), stop=(k == KT - 1))
                    src = ps.rearrange("p (h w) -> p h w", w=64)
                    dst = v_aug[:, it, :].rearrange("p (h w) -> p h w", w=65)
                    nc.vector.tensor_copy(dst[:, :, 0:64], src)

        # allocated after the projection pool closes so phase-1 SBUF peak
        # (xt/wq/wv tiles) and these never coexist in the address map
        p2 = tc.alloc_tile_pool(name="persist2", bufs=1)
        ot = [p2.tile([128, N], bf16, tag=f"ot{t}", name=f"ot{t}")
              for t in range(4)]
        wo_sb = [p2.tile([128, D], bf16, tag=f"wo{t}", name=f"wo{t}")
                 for t in range(4)]
        for t in range(4):
            nc.sync.dma_start(out=wo_sb[t], in_=wo_ap[t * 128 : (t + 1) * 128, :])

        # ---- phase 2+3: attention (ic outer) with overlapped output proj ----
        with tc.tile_pool(name="sps", bufs=2, space="PSUM") as sps, \
             tc.tile_pool(name="nps", bufs=2, space="PSUM") as nps, \
             tc.tile_pool(name="gp", bufs=3) as gp, \
             tc.tile_pool(name="nrm", bufs=2) as nrm, \
             tc.tile_pool(name="osb", bufs=2) as osb:

            def emit_oproj(it):
                isl = slice(it * 128, (it + 1) * 128)
                # same tag+shape as the scores tiles: shares the sps PSUM
                # ring (a separate tag would blow the 8-bank budget)
                ps = sps.tile([128, ICW], f32, tag="scores")
                for ock in range(2):
                    osl = slice(ock * 512, (ock + 1) * 512)
                    for dlt in range(4):
                        nc.tensor.matmul(ps[:, osl], lhsT=ot[dlt][:, isl],
                                         rhs=wo_sb[dlt][:, osl],
                                         start=(dlt == 0), stop=(dlt == 3))
                ob = osb.tile([128, 1024], f32, tag="obuf", name="obuf")
                nc.vector.tensor_copy(ob, ps)
                nc.sync.dma_start(out=part[isl, :], in_=ob)

            pending = []
            for ic in range(IC):
                i0 = ic * ICW
                for h in range(NH):
                    dt, half = divmod(h, 2)
                    rows = slice(half * 64, half * 64 + 64)
                    vsl = slice(h * 65, (h + 1) * 65)
                    nm = nps.tile([65, ICW], f32, tag="num")
                    gs = [None] * JT
                    for jt in range(JT):
                        jsl = slice(jt * 128, (jt + 1) * 128)
                        sp = sps.tile([128, ICW], f32, tag="scores")
                        for q in range(2):
                            qsl = slice(q * 512, (q + 1) * 512)
                            nc.tensor.matmul(
                                sp[:, qsl], lhsT=qS[h][:, jsl],
                                rhs=qM[h][:, i0 + q * 512 : i0 + (q + 1) * 512],
                                start=True, stop=True)
                        g = gp.tile([128, ICW], f16, tag="gtile")
                        nc.scalar.activation(out=g, in_=sp, func=EXP,
                                             scale=2.0 * SCALE)
                        gs[jt] = g
                        # one-step software skew: num(jt-1) after S(jt)/exp(jt)
                        if jt > 0:
                            for q in range(2):
                                qsl = slice(q * 512, (q + 1) * 512)
                                nc.tensor.matmul(nm[:, qsl], lhsT=v_aug[:, jt - 1, vsl],
                                                 rhs=gs[jt - 1][:, qsl],
                                                 start=(jt == 1), stop=False)
                        # spread the previous chunk's output proj through
                        # this chunk's first head
                        if pending and jt % 2 == 1:
                            emit_oproj(pending.pop(0))
                    for q in range(2):
                        qsl = slice(q * 512, (q + 1) * 512)
                        nc.tensor.matmul(nm[:, qsl], lhsT=v_aug[:, JT - 1, vsl],
                                         rhs=gs[JT - 1][:, qsl],
                                         start=False, stop=True)
                    # normalize: ot[rows, i0:i0+ICW] = nm[0:64] / nm[64]
                    # (reciprocal_approx_fast misreads PSUM - stage via SBUF)
                    dsb = nrm.tile([1, ICW], f32, tag="dsb", name="dsb")
                    nc.vector.tensor_copy(dsb, nm[64:65, :])
                    rd = nrm.tile([1, ICW], f32, tag="rden", name="rden")
                    nc.vector.reciprocal_approx_fast(out=rd, in_=dsb)
                    rdb = nrm.tile([64, ICW], f32, tag="rdenb", name="rdenb")
                    nc.gpsimd.partition_broadcast(rdb, rd)
                    nc.vector.tensor_mul(ot[dt][rows, i0 : i0 + ICW],
                                         nm[0:64, :], rdb)
                pending = list(range(ic * JT // IC, (ic + 1) * JT // IC))
            for it in pending:
                emit_oproj(it)

        p2.release()

    nc.compile()
    return nc


def _get_nc():
    if "nc" not in _CACHE:
        _CACHE["nc"] = _build_nc()
    return _CACHE["nc"]


def make_in_maps(x, Wq, bq, Wv, bv, Wo, bo):
    from ml_dtypes import bfloat16

    x = np.asarray(x, dtype=np.float32)
    Wq = np.asarray(Wq, dtype=np.float32)
    Wv = np.asarray(Wv, dtype=np.float32)
    Wo = np.asarray(Wo, dtype=np.float32)
    in_maps = []
    for c in range(NCORES):
        b, g = divmod(c, 2)
        gsl = slice(g * DL, (g + 1) * DL)
        in_maps.append({
            "xTa": np.ascontiguousarray(x[b].T).astype(bfloat16),
            "wqa": np.ascontiguousarray(Wq[:, gsl]).astype(bfloat16),
            "wva": np.ascontiguousarray(Wv[:, gsl]).astype(bfloat16),
            "wo": np.ascontiguousarray(Wo[gsl, :]).astype(bfloat16),
        })
    return in_maps


def combine_parts(parts, bv, Wo, bo):
    bo = np.asarray(bo, dtype=np.float32)
    bv = np.asarray(bv, dtype=np.float32)
    Wo = np.asarray(Wo, dtype=np.float32)
    bias = bo + bv @ Wo  # v-bias commutes through the softmax average
    out = np.empty((B, N, D), np.float32)
    for b in range(B):
        out[b] = parts[2 * b] + parts[2 * b + 1] + bias
    return out


def kernel(x, Wq, bq, Wv, bv, Wo, bo):
    from concourse.bass_utils import run_bass_kernel_spmd

    nc = _get_nc()
    in_maps = make_in_maps(x, Wq, bq, Wv, bv, Wo, bo)
    res = run_bass_kernel_spmd(nc, in_maps, core_ids=list(range(NCORES)))
    parts = [r["part"] for r in res.results]
    return combine_parts(parts, bv, Wo, bo)
